# revision 2
# baseline (speedup 1.0000x reference)
"""Trainium2 Bass kernel for FMGCNCell (adaptive-graph GRU cell), v2.

Key change vs v1: per-call host->device staging dominates (≈0.53 ms/MB/core),
so each core now ships only its own node slab (~3 MB) and the full X matrix
is assembled on-device via AllGather. All transposes/layout shuffles happen
on-chip; output returns in bf16.

Sharding: node-parallel (each core owns N/8 = 250 output nodes, full batch).
"""

from contextlib import ExitStack

import numpy as np
import ml_dtypes

import concourse.bass as bass
import concourse.bacc as bacc
import concourse.tile as tile
from concourse import mybir
from concourse.bass import ds, ts
from concourse.bass_utils import run_bass_kernel_spmd
from concourse.masks import make_identity

F32 = mybir.dt.float32
BF16 = mybir.dt.bfloat16
AF = mybir.ActivationFunctionType
ALU = mybir.AluOpType

# Problem constants
B = 64
DIN = 2
H = 64
E = 16
CAT = DIN + H            # 66
KI0 = CAT + 1            # 67 rows: 66 inputs + 1 bias row
KIALL = 2 * CAT + 1      # 133
OG = 2 * H               # 128 gate outputs (z|r)
OU = H                   # 64 update outputs
BC = B * CAT             # 4224; X layout is (c, b) c-major
N = 2000
NC_ = 8
NOWN = N // NC_          # 250
NB = B * NOWN            # 16000

CA = 32 * B              # 2048 cols (c 0:32) of X for AllGather half A
CB = BC - CA             # 2176 cols (c 32:66)
ZH = 32 * B              # 2048 cols per zs half (h 0:32 / 32:64)


def _chunks(total, size):
    out = []
    off = 0
    while off < total:
        out.append((off, min(size, total - off)))
        off += size
    return out


def build_nc(n_cores=8, nblk=20):
    mch = _chunks(N, 128)          # 16 chunks of the contraction/node axis
    MC = len(mch)

    nc = bacc.Bacc("TRN2", target_bir_lowering=False, debug=False,
                   num_devices=n_cores)

    # ---- external inputs ----
    xt_d = nc.dram_tensor("xt_in", [KI0, NB], BF16, kind="ExternalInput")
    eT_all_d = nc.dram_tensor("eT_all", [E, N], F32, kind="ExternalInput")
    eT_own_d = nc.dram_tensor("eT_own", [E, NOWN], F32, kind="ExternalInput")
    wpg_d = nc.dram_tensor("wpg", [E, KIALL, OG], F32, kind="ExternalInput")
    wpu_d = nc.dram_tensor("wpu", [E, KIALL, OU], F32, kind="ExternalInput")

    # output: [h, (n, b)] n-major, bf16 (host reassembles)
    out_d = nc.dram_tensor("out", [H, NB], BF16, kind="ExternalOutput")

    # ---- internal DRAM ----
    x1_own_d = nc.dram_tensor("x1_own", [NOWN, BC], BF16)
    zs_own_d = nc.dram_tensor("zs_own", [NOWN, H * B], BF16)
    wg_hi_d = nc.dram_tensor("wg_hi_d", [NOWN, KIALL, OG], BF16)
    wg_lo_d = nc.dram_tensor("wg_lo_d", [NOWN, KIALL, OG], BF16)
    wu_hi_d = nc.dram_tensor("wu_hi_d", [NOWN, KIALL, OU], BF16)
    wu_lo_d = nc.dram_tensor("wu_lo_d", [NOWN, KIALL, OU], BF16)
    xg_d = nc.dram_tensor("xg_d", [BC, NOWN], BF16)
    xg_lo_d = nc.dram_tensor("xg_lo_d", [BC, NOWN], BF16)
    r_d = nc.dram_tensor("r_d", [NOWN, H * B], BF16)
    shared = dict(addr_space="Shared") if n_cores > 1 else {}
    x1_all_d = nc.dram_tensor("x1_all", [N, BC], BF16, **shared)
    zs_all_d = nc.dram_tensor("zs_all", [N, H * B], BF16, **shared)

    with tile.TileContext(nc) as tc:
        with ExitStack() as root:
            persist = root.enter_context(tc.tile_pool(name="persist", bufs=1))
            XT = persist.tile([KI0, NB], BF16)           # [c, (b, n)] b-major
            M_sb = persist.tile([128, MC * NOWN], BF16)  # support columns
            rinv_bc = persist.tile([128, NOWN], F32)
            xcols = persist.tile([128, MC * 128], BF16)  # X cols 4096:4224
            XG = persist.tile([CAT, NB], BF16)           # xg1T then xg2T, [c, (b, n)]
            XG_lo = persist.tile([CAT, NB], BF16)        # bf16 residual of xg
            eT_own = persist.tile([E, NOWN], F32)
            ident = persist.tile([128, 128], BF16)
            make_identity(nc, ident[:])
            XB = persist.tile([3, NB], BF16)     # x|ones rows at partition 0

            # ---------- P0: load XT ----------
            nc.sync.dma_start(XT[:], xt_d[:])
            nc.sync.dma_start(XB[:], xt_d[H:KI0, :])
            nc.sync.dma_start(eT_own[:], eT_own_d[:])

            # ---------- P2: build x1_own from XT by transposes ----------
            with ExitStack() as p2:
                tps = p2.enter_context(tc.tile_pool(name="tps", bufs=3, space="PSUM"))
                stg = p2.enter_context(tc.tile_pool(name="stg", bufs=1))
                S0 = stg.tile([128, BC], BF16)
                S1 = stg.tile([128, BC], BF16)
                for b in range(B):
                    for half, S in ((0, S0), (1, S1)):
                        ps = tps.tile([128, 128], BF16, tag="tps")
                        nc.tensor.transpose(
                            ps[:125, :CAT],
                            XT[:CAT, b * NOWN + half * 125:
                               b * NOWN + half * 125 + 125],
                            ident[:CAT, :CAT])
                        # scatter into (c,b)-major columns: col = c*64 + b
                        nc.vector.tensor_copy(
                            S[:125, :].rearrange("n (c b2) -> n c b2", b2=B)[:, :, b],
                            ps[:125, :CAT])
                nc.sync.dma_start(x1_own_d[:125, :], S0[:125, :])
                nc.sync.dma_start(x1_own_d[125:, :], S1[:125, :])

            # ---------- P3: AllGather x1 ----------
            if n_cores > 1:
                nc.gpsimd.collective_compute(
                    "AllGather", ALU.bypass,
                    replica_groups=[list(range(n_cores))],
                    ins=[x1_own_d[:]], outs=[x1_all_d[:]])
            else:
                nc.sync.dma_start(x1_all_d[:], x1_own_d[:])

            # ---------- P1: adaptive support M and row-sum ----------
            with ExitStack() as p1:
                eT_pool = p1.enter_context(tc.tile_pool(name="eT", bufs=1))
                eT_all = eT_pool.tile([E, N], F32)
                nc.sync.dma_start(eT_all[:], eT_all_d[:])
                sm_pool = p1.enter_context(tc.tile_pool(name="sm", bufs=3))
                sm_psum = p1.enter_context(tc.tile_pool(name="sm_ps", bufs=2, space="PSUM"))
                for j, (m0, mp) in enumerate(mch):
                    ps = sm_psum.tile([128, NOWN], F32, tag="sm_ps")
                    nc.tensor.matmul(ps[:mp, :], eT_all[:, m0:m0 + mp],
                                     eT_own[:, :], start=True, stop=True)
                    ex = sm_pool.tile([128, NOWN], F32, tag="sm_ex")
                    nc.scalar.activation(ex[:mp, :], ps[:mp, :], AF.Exp)
                    nc.vector.tensor_scalar_max(M_sb[:mp, ts(j, NOWN)], ex[:mp, :],
                                                1.0)
                ones_pool = p1.enter_context(tc.tile_pool(name="ones", bufs=1))
                ones = ones_pool.tile([128, 1], BF16)
                nc.vector.memset(ones[:], 1.0)
                rs_ps = sm_psum.tile([1, NOWN], F32, tag="rs_ps")
                for j, (m0, mp) in enumerate(mch):
                    nc.tensor.matmul(rs_ps[:, :], ones[:mp, :],
                                     M_sb[:mp, ts(j, NOWN)],
                                     start=(j == 0), stop=(j == MC - 1))
                rinv = sm_pool.tile([1, NOWN], F32, tag="rinv")
                nc.vector.reciprocal(rinv[:, :], rs_ps[:, :])
                nc.gpsimd.partition_broadcast(rinv_bc[:], rinv[:, :])

            # ---------- P3b: W-gen (f32) -> bf16 hi/lo pairs in DRAM ----------
            nch = _chunks(NOWN, 128)
            with ExitStack() as p3:
                wgen_rhs = p3.enter_context(tc.tile_pool(name="wg_rhs", bufs=3))
                wgen_ps = p3.enter_context(tc.tile_pool(name="wg_ps", bufs=2, space="PSUM"))
                wgen_pool = p3.enter_context(tc.tile_pool(name="wg_ev", bufs=4))
                for (wdram_in, whi, wlo, O) in (
                        (wpg_d, wg_hi_d, wg_lo_d, OG),
                        (wpu_d, wu_hi_d, wu_lo_d, OU)):
                    KO = KIALL * O
                    for (f0, fp) in _chunks(KO, 512):
                        rhs = wgen_rhs.tile([E, 512], F32, tag="wg_rhs")
                        nc.sync.dma_start(
                            rhs[:, :fp],
                            wdram_in[:].rearrange("e k o -> e (k o)")[:, f0:f0 + fp])
                        for (nn0, np_) in nch:
                            ps = wgen_ps.tile([128, 512], F32, tag="wg_ps")
                            nc.tensor.matmul(ps[:np_, :fp],
                                             eT_own[:, nn0:nn0 + np_],
                                             rhs[:, :fp],
                                             start=True, stop=True)
                            ev = wgen_pool.tile([128, 512], BF16, tag="wg_ev")
                            nc.vector.tensor_copy(ev[:np_, :fp], ps[:np_, :fp])
                            nc.sync.dma_start(
                                whi[:].rearrange("n k o -> n (k o)")
                                [nn0:nn0 + np_, f0:f0 + fp],
                                ev[:np_, :fp])
                            el = wgen_pool.tile([128, 512], BF16, tag="wg_el")
                            nc.vector.tensor_sub(el[:np_, :fp], ps[:np_, :fp],
                                                 ev[:np_, :fp])
                            nc.sync.dma_start(
                                wlo[:].rearrange("n k o -> n (k o)")
                                [nn0:nn0 + np_, f0:f0 + fp],
                                el[:np_, :fp])

            # ---------- conv helper: (1/r) * A @ X -> xg_d -> XG ----------
            def conv(parts, is_first):
                with ExitStack() as pc:
                    xg_pool = pc.enter_context(tc.tile_pool(name="xgrp", bufs=2))
                    cv_ps = pc.enter_context(tc.tile_pool(name="cv_ps", bufs=5, space="PSUM"))
                    cv_ev = pc.enter_context(tc.tile_pool(name="cv_ev", bufs=3))

                    def emit_q(q, lhs_tile, qi, gcols):
                        ps = cv_ps.tile([128, NOWN], F32, tag="cv_ps")
                        for j, (m0, mp) in enumerate(mch):
                            nc.tensor.matmul(
                                ps[:128, :],
                                lhs_tile[:mp, ts(j, gcols)][:, qi * 128:(qi + 1) * 128]
                                if gcols else lhs_tile[:mp, ts(j, 128)],
                                M_sb[:mp, ts(j, NOWN)],
                                start=(j == 0), stop=(j == MC - 1))
                        evf = cv_ev.tile([128, NOWN], F32, tag="cv_evf")
                        nc.vector.tensor_tensor(evf[:, :], ps[:, :],
                                                rinv_bc[:, :], ALU.mult)
                        ev = cv_ev.tile([128, NOWN], BF16, tag="cv_ev")
                        nc.vector.tensor_copy(ev[:, :], evf[:, :])
                        nc.sync.dma_start(xg_d[q * 128:(q + 1) * 128, :],
                                          ev[:, :])
                        el = cv_ev.tile([128, NOWN], BF16, tag="cv_el")
                        nc.vector.tensor_sub(el[:, :], evf[:, :], ev[:, :])
                        nc.sync.dma_start(xg_lo_d[q * 128:(q + 1) * 128, :],
                                          el[:, :])

                    for (dram, g0, gcols, qoffs) in parts:
                        if dram is None:
                            emit_q(qoffs[0], xcols, 0, 0)
                            continue
                        Xg = xg_pool.tile([128, MC * 5 * 128], BF16, tag="xgrp")
                        for j, (m0, mp) in enumerate(mch):
                            nc.sync.dma_start(Xg[:mp, ts(j, gcols)],
                                              dram[m0:m0 + mp, g0:g0 + gcols])
                            if is_first and g0 + gcols == BC:
                                nc.vector.tensor_copy(
                                    xcols[:mp, ts(j, 128)],
                                    Xg[:mp, ts(j, gcols)][:, gcols - 128:])
                        for qi, q in enumerate(qoffs):
                            emit_q(q, Xg, qi, gcols)
                    # strided reloads: [(c,b), n] -> [c, (b, n)]
                    nc.sync.dma_start(
                        XG[:].rearrange("c (b2 n) -> c b2 n", b2=B),
                        xg_d[:].rearrange("(c b2) n -> c b2 n", b2=B))
                    nc.sync.dma_start(
                        XG_lo[:].rearrange("c (b2 n) -> c b2 n", b2=B),
                        xg_lo_d[:].rearrange("(c b2) n -> c b2 n", b2=B))

            def groups(q0, nq, cap=5):
                out = []
                q = q0
                while q < q0 + nq:
                    take = min(cap, q0 + nq - q)
                    out.append(list(range(q, q + take)))
                    q += take
                return out

            # ---------- P4: conv1 -> XG = xg1T ----------
            parts1 = [(x1_all_d, qs[0] * 128, len(qs) * 128, qs)
                      for qs in groups(0, 33)]
            conv(parts1, is_first=True)

            # ---------- P5: apply gate; spill z*state ----------
            with ExitStack() as p5:
                ap_w = p5.enter_context(tc.tile_pool(name="ap_w", bufs=2))
                ap_ps = p5.enter_context(tc.tile_pool(name="ap_ps", bufs=3, space="PSUM"))
                ap_t = p5.enter_context(tc.tile_pool(name="ap_t", bufs=3))
                for (nb0, nbp) in _chunks(NOWN, nblk):
                    w0h = ap_w.tile([KI0, nblk * OG], BF16, tag="w0h")
                    nc.sync.dma_start(
                        w0h[:, :nbp * OG].rearrange("k (n o) -> k n o", o=OG),
                        wg_hi_d[nb0:nb0 + nbp, :KI0, :].rearrange("n k o -> k n o"))
                    w0l = ap_w.tile([KI0, nblk * OG], BF16, tag="w0l")
                    nc.sync.dma_start(
                        w0l[:, :nbp * OG].rearrange("k (n o) -> k n o", o=OG),
                        wg_lo_d[nb0:nb0 + nbp, :KI0, :].rearrange("n k o -> k n o"))
                    w1h = ap_w.tile([CAT, nblk * OG], BF16, tag="w1h")
                    nc.sync.dma_start(
                        w1h[:, :nbp * OG].rearrange("k (n o) -> k n o", o=OG),
                        wg_hi_d[nb0:nb0 + nbp, KI0:, :].rearrange("n k o -> k n o"))
                    w1l = ap_w.tile([CAT, nblk * OG], BF16, tag="w1l")
                    nc.sync.dma_start(
                        w1l[:, :nbp * OG].rearrange("k (n o) -> k n o", o=OG),
                        wg_lo_d[nb0:nb0 + nbp, KI0:, :].rearrange("n k o -> k n o"))
                    for (g0, gp) in _chunks(nbp, 8):
                        ps_z = ap_ps.tile([H, 512], F32, tag="ap_psz")
                        ps_r = ap_ps.tile([H, 512], F32, tag="ap_psr")
                        for nl in range(g0, g0 + gp):
                            n = nb0 + nl
                            w = (nl - g0) * B
                            xt_n = XT[:, n::NOWN][:, :B]
                            xgh_n = XG[:, n::NOWN][:, :B]
                            xgl_n = XG_lo[:, n::NOWN][:, :B]
                            for ps, o0 in ((ps_z, 0), (ps_r, H)):
                                sl = slice(nl * OG + o0, nl * OG + o0 + H)
                                nc.tensor.matmul(ps[:, w:w + B], w0h[:, sl],
                                                 xt_n, start=True, stop=False)
                                nc.tensor.matmul(ps[:, w:w + B], w0l[:, sl],
                                                 xt_n, start=False, stop=False)
                                nc.tensor.matmul(ps[:, w:w + B], w1h[:CAT, sl],
                                                 xgh_n, start=False, stop=False)
                                nc.tensor.matmul(ps[:, w:w + B], w1h[:CAT, sl],
                                                 xgl_n, start=False, stop=False)
                                nc.tensor.matmul(ps[:, w:w + B], w1l[:CAT, sl],
                                                 xgh_n, start=False, stop=True)
                        cols = slice((nb0 + g0) * B, (nb0 + g0 + gp) * B)
                        zg = ap_t.tile([H, 512], F32, tag="zg")
                        nc.scalar.activation(zg[:, :gp * B], ps_z[:, :gp * B],
                                             AF.Sigmoid)
                        rg = ap_t.tile([H, 512], BF16, tag="rg")
                        nc.scalar.activation(rg[:, :gp * B], ps_r[:, :gp * B],
                                             AF.Sigmoid)
                        nc.sync.dma_start(
                            r_d[nb0 + g0:nb0 + g0 + gp, :]
                            .rearrange("n (h b2) -> h n b2", b2=B),
                            rg[:, :gp * B].rearrange("h (n b2) -> h n b2", b2=B))
                        zs = ap_t.tile([H, 512], BF16, tag="zs")
                        nc.vector.tensor_tensor(
                            zs[:, :gp * B].rearrange("h (n b2) -> h n b2", b2=B),
                            zg[:, :gp * B].rearrange("h (n b2) -> h n b2", b2=B),
                            XT[:H, :].rearrange("h (b2 n) -> h n b2", b2=B)
                            [:, nb0 + g0:nb0 + g0 + gp, :],
                            ALU.mult)
                        nc.sync.dma_start(
                            zs_own_d[nb0 + g0:nb0 + g0 + gp, :]
                            .rearrange("n (h b2) -> h n b2", b2=B),
                            zs[:, :gp * B].rearrange("h (n b2) -> h n b2", b2=B))

            # ---------- P6: AllGather z*state ----------
            if n_cores > 1:
                nc.gpsimd.collective_compute(
                    "AllGather", ALU.bypass,
                    replica_groups=[list(range(n_cores))],
                    ins=[zs_own_d[:]], outs=[zs_all_d[:]])
            else:
                nc.sync.dma_start(zs_all_d[:], zs_own_d[:])

            # ---------- P7: conv2 -> XG = xg2T ----------
            parts2 = [(zs_all_d, qs[0] * 128, len(qs) * 128, qs)
                      for qs in groups(0, 32)]
            parts2.append((None, 32 * 128, 128, [32]))
            conv(parts2, is_first=False)

            # ---------- P8: apply update; blend; output ----------
            with ExitStack() as p8:
                ap_w = p8.enter_context(tc.tile_pool(name="ap_w2", bufs=2))
                ap_ps = p8.enter_context(tc.tile_pool(name="ap_ps2", bufs=3, space="PSUM"))
                ap_t = p8.enter_context(tc.tile_pool(name="ap_t2", bufs=3))
                zst_pool = p8.enter_context(tc.tile_pool(name="zstb", bufs=2))
                for (nb0, nbp) in _chunks(NOWN, nblk):
                    w0ah = ap_w.tile([H, nblk * OU], BF16, tag="w0ah")
                    nc.sync.dma_start(
                        w0ah[:, :nbp * OU].rearrange("k (n o) -> k n o", o=OU),
                        wu_hi_d[nb0:nb0 + nbp, :H, :].rearrange("n k o -> k n o"))
                    w0al = ap_w.tile([H, nblk * OU], BF16, tag="w0al")
                    nc.sync.dma_start(
                        w0al[:, :nbp * OU].rearrange("k (n o) -> k n o", o=OU),
                        wu_lo_d[nb0:nb0 + nbp, :H, :].rearrange("n k o -> k n o"))
                    w0bh = ap_w.tile([3, nblk * OU], BF16, tag="w0bh")
                    nc.sync.dma_start(
                        w0bh[:, :nbp * OU].rearrange("k (n o) -> k n o", o=OU),
                        wu_hi_d[nb0:nb0 + nbp, H:KI0, :].rearrange("n k o -> k n o"))
                    w0bl = ap_w.tile([3, nblk * OU], BF16, tag="w0bl")
                    nc.sync.dma_start(
                        w0bl[:, :nbp * OU].rearrange("k (n o) -> k n o", o=OU),
                        wu_lo_d[nb0:nb0 + nbp, H:KI0, :].rearrange("n k o -> k n o"))
                    w1h = ap_w.tile([CAT, nblk * OU], BF16, tag="w1uh")
                    nc.sync.dma_start(
                        w1h[:, :nbp * OU].rearrange("k (n o) -> k n o", o=OU),
                        wu_hi_d[nb0:nb0 + nbp, KI0:, :].rearrange("n k o -> k n o"))
                    w1l = ap_w.tile([CAT, nblk * OU], BF16, tag="w1ul")
                    nc.sync.dma_start(
                        w1l[:, :nbp * OU].rearrange("k (n o) -> k n o", o=OU),
                        wu_lo_d[nb0:nb0 + nbp, KI0:, :].rearrange("n k o -> k n o"))
                    zst = zst_pool.tile([H, nblk * B], BF16, tag="zstb")
                    nc.sync.dma_start(
                        zst[:, :nbp * B].rearrange("h (n b2) -> h n b2", b2=B),
                        zs_own_d[nb0:nb0 + nbp, :]
                        .rearrange("n (h b2) -> h n b2", b2=B))
                    rb = zst_pool.tile([H, nblk * B], BF16, tag="rb")
                    nc.sync.dma_start(
                        rb[:, :nbp * B].rearrange("h (n b2) -> h n b2", b2=B),
                        r_d[nb0:nb0 + nbp, :]
                        .rearrange("n (h b2) -> h n b2", b2=B))
                    for (g0, gp) in _chunks(nbp, 8):
                        ps = ap_ps.tile([H, 512], F32, tag="ap_ps2")
                        for nl in range(g0, g0 + gp):
                            n = nb0 + nl
                            w = (nl - g0) * B
                            zst_n = zst[:, nl * B:(nl + 1) * B]
                            xb_n = XB[:, n::NOWN][:, :B]
                            xgh_n = XG[:, n::NOWN][:, :B]
                            xgl_n = XG_lo[:, n::NOWN][:, :B]
                            sl = slice(nl * OU, (nl + 1) * OU)
                            nc.tensor.matmul(ps[:, w:w + B], w0ah[:, sl],
                                             zst_n, start=True, stop=False)
                            nc.tensor.matmul(ps[:, w:w + B], w0al[:, sl],
                                             zst_n, start=False, stop=False)
                            nc.tensor.matmul(ps[:, w:w + B], w0bh[:, sl],
                                             xb_n, start=False, stop=False)
                            nc.tensor.matmul(ps[:, w:w + B], w0bl[:, sl],
                                             xb_n, start=False, stop=False)
                            nc.tensor.matmul(ps[:, w:w + B], w1h[:CAT, sl],
                                             xgh_n, start=False, stop=False)
                            nc.tensor.matmul(ps[:, w:w + B], w1h[:CAT, sl],
                                             xgl_n, start=False, stop=False)
                            nc.tensor.matmul(ps[:, w:w + B], w1l[:CAT, sl],
                                             xgh_n, start=False, stop=True)
                        cols = slice((nb0 + g0) * B, (nb0 + g0 + gp) * B)
                        hc = ap_t.tile([H, 512], F32, tag="hc")
                        nc.scalar.activation(hc[:, :gp * B], ps[:, :gp * B],
                                             AF.Tanh)
                        # out = hc + r*(state - hc)
                        t1 = ap_t.tile([H, 512], F32, tag="t1")
                        nc.vector.tensor_sub(
                            t1[:, :gp * B].rearrange("h (n b2) -> h n b2", b2=B),
                            XT[:H, :].rearrange("h (b2 n) -> h n b2", b2=B)
                            [:, nb0 + g0:nb0 + g0 + gp, :],
                            hc[:, :gp * B].rearrange("h (n b2) -> h n b2", b2=B))
                        t2 = ap_t.tile([H, 512], F32, tag="t2")
                        nc.vector.tensor_tensor(
                            t2[:, :gp * B], t1[:, :gp * B],
                            rb[:, cols.start - nb0 * B:cols.stop - nb0 * B],
                            ALU.mult)
                        ot = ap_t.tile([H, 512], BF16, tag="ot")
                        nc.vector.tensor_add(ot[:, :gp * B], t2[:, :gp * B],
                                             hc[:, :gp * B])
                        nc.sync.dma_start(out_d[:, cols], ot[:, :gp * B])

    nc.compile()
    return nc


_NC_CACHE = {}


def _get_nc(n_cores=8):
    if n_cores not in _NC_CACHE:
        _NC_CACHE[n_cores] = build_nc(n_cores=n_cores)
    return _NC_CACHE[n_cores]


def _pack_pool(wp, bias, O):
    """[E,K,CAT,O] pool + [E,O] bias -> [E, 133, O] bf16.

    Row order per k-slab: (state rows, x rows); bias at row 66."""
    out = np.empty((E, KIALL, O), np.float32)
    out[:, :H, :] = wp[:, 0, DIN:, :]
    out[:, H:CAT, :] = wp[:, 0, :DIN, :]
    out[:, CAT, :] = bias
    out[:, KI0:KI0 + H, :] = wp[:, 1, DIN:, :]
    out[:, KI0 + H:, :] = wp[:, 1, :DIN, :]
    return out


def _build_in_maps(x, state, node_embed, gate_weights_pool, gate_bias_pool,
                   update_weights_pool, update_bias_pool, n_cores=8):
    x = np.asarray(x, np.float32)
    state = np.asarray(state, np.float32)
    node_embed = np.asarray(node_embed, np.float32)
    eT = np.ascontiguousarray(node_embed.T)                 # [E, N]
    x1_nbc = np.concatenate([state.transpose(1, 0, 2), x.transpose(1, 0, 2)],
                            axis=2)                          # [N, B, CAT]
    wpg = _pack_pool(np.asarray(gate_weights_pool, np.float32),
                     np.asarray(gate_bias_pool, np.float32), OG)
    wpu = _pack_pool(np.asarray(update_weights_pool, np.float32),
                     np.asarray(update_bias_pool, np.float32), OU)
    in_maps = []
    for c in range(n_cores):
        sl = slice(c * NOWN, (c + 1) * NOWN)
        eT_own = np.ascontiguousarray(eT[:, sl])
        xt = np.ones((KI0, B, NOWN), np.float32)
        xt[:CAT] = x1_nbc[sl].transpose(2, 1, 0)
        in_maps.append({
            "xt_in": xt.astype(ml_dtypes.bfloat16).reshape(KI0, NB),
            "eT_all": eT,
            "eT_own": eT_own,
            "wpg": wpg,
            "wpu": wpu,
        })
    return in_maps


def kernel(x, state, node_embed, gate_weights_pool, gate_bias_pool,
           update_weights_pool, update_bias_pool, n_cores=8):
    nc = _get_nc(n_cores)
    in_maps = _build_in_maps(x, state, node_embed, gate_weights_pool,
                             gate_bias_pool, update_weights_pool,
                             update_bias_pool, n_cores)
    res = run_bass_kernel_spmd(nc, in_maps, list(range(n_cores)))
    outs = []
    for c in range(n_cores):
        o = np.asarray(res.results[c]["out"], dtype=np.float32)  # [H, NOWN*B]
        outs.append(o.reshape(H, NOWN, B).transpose(2, 1, 0))    # [B, NOWN, H]
    return np.concatenate(outs, axis=1)


# revision 3
# speedup vs baseline: 1.0033x; 1.0033x over previous
"""Trainium2 Bass kernel for FMGCNCell (adaptive-graph GRU cell), v3 (fp16).

Per-call host->device staging dominates (~0.53 ms/MB/core), so each core
ships only its own node slab (~3 MB) and the full X matrix is assembled
on-device via AllGather. The whole datapath runs in fp16 (10-bit mantissa)
with f32 PSUM accumulation; the adaptive support is pre-normalized so its
fp16 copy stays in [0, 1]. Output returns fp16, host converts.

Sharding: node-parallel (each core owns N/8 = 250 output nodes, full batch).
"""

from contextlib import ExitStack

import numpy as np
import ml_dtypes

import concourse.bass as bass
import concourse.bacc as bacc
import concourse.tile as tile
from concourse import mybir
from concourse.bass import ds, ts
from concourse.bass_utils import run_bass_kernel_spmd
from concourse.masks import make_identity

F32 = mybir.dt.float32
BF16 = mybir.dt.bfloat16
F16 = mybir.dt.float16
AF = mybir.ActivationFunctionType
ALU = mybir.AluOpType

B = 64
DIN = 2
H = 64
E = 16
CAT = DIN + H            # 66
KI0 = CAT + 1            # 67
KIALL = 2 * CAT + 1      # 133
OG = 2 * H               # 128
OU = H                   # 64
BC = B * CAT             # 4224; X layout (c, b) c-major
N = 2000
NC_ = 8
NOWN = N // NC_          # 250
NB = B * NOWN            # 16000


def _chunks(total, size):
    out = []
    off = 0
    while off < total:
        out.append((off, min(size, total - off)))
        off += size
    return out


def build_nc(n_cores=8, nblk=25):
    mch = _chunks(N, 128)
    MC = len(mch)

    nc = bacc.Bacc("TRN2", target_bir_lowering=False, debug=False,
                   num_devices=n_cores)

    # ---- external inputs ----
    xt_d = nc.dram_tensor("xt_in", [KI0, NB], F16, kind="ExternalInput")
    eT_all_d = nc.dram_tensor("eT_all", [E, N], F32, kind="ExternalInput")
    eT_own_d = nc.dram_tensor("eT_own", [E, NOWN], F32, kind="ExternalInput")
    wpg_d = nc.dram_tensor("wpg", [E, KIALL, OG], F16, kind="ExternalInput")
    wpu_d = nc.dram_tensor("wpu", [E, KIALL, OU], F16, kind="ExternalInput")

    # output: [h, (n, b)] n-major, fp16 (host reassembles)
    out_d = nc.dram_tensor("out", [H, NB], F16, kind="ExternalOutput")

    # ---- internal DRAM ----
    x1_own_d = nc.dram_tensor("x1_own", [NOWN, BC], F16)
    zs_own_d = nc.dram_tensor("zs_own", [NOWN, H * B], F16)
    wg_d = nc.dram_tensor("wg_d", [NOWN, KIALL, OG], F16)
    wu_d = nc.dram_tensor("wu_d", [NOWN, KIALL, OU], F16)
    xg_d = nc.dram_tensor("xg_d", [BC, NOWN], F16)
    r_d = nc.dram_tensor("r_d", [NOWN, H * B], F16)
    shared = dict(addr_space="Shared") if n_cores > 1 else {}
    x1_all_d = nc.dram_tensor("x1_all", [N, BC], F16, **shared)
    zs_all_d = nc.dram_tensor("zs_all", [N, H * B], F16, **shared)

    with tile.TileContext(nc) as tc:
        with ExitStack() as root:
            persist = root.enter_context(tc.tile_pool(name="persist", bufs=1))
            XT = persist.tile([KI0, NB], F16)            # [c, (b, n)] b-major
            A_sb = persist.tile([128, MC * NOWN], F16)   # normalized support
            xcols = persist.tile([128, MC * 128], F16)   # X cols 4096:4224
            XG = persist.tile([CAT, NB], F16)            # xg1T then xg2T
            ident = persist.tile([128, 128], F16)
            make_identity(nc, ident[:])
            XB = persist.tile([3, NB], F16)              # x|ones rows at part 0
            eT_own = persist.tile([E, NOWN], F32)
            eT16 = persist.tile([E, NOWN], F16)

            # ---------- P0: load inputs ----------
            nc.sync.dma_start(XT[:], xt_d[:])
            nc.sync.dma_start(XB[:], xt_d[H:KI0, :])
            nc.sync.dma_start(eT_own[:], eT_own_d[:])
            nc.vector.tensor_copy(eT16[:], eT_own[:])

            # ---------- P2: build x1_own from XT by transposes ----------
            with ExitStack() as p2:
                tps = p2.enter_context(tc.tile_pool(name="tps", bufs=3, space="PSUM"))
                stg = p2.enter_context(tc.tile_pool(name="stg", bufs=1))
                S0 = stg.tile([128, BC], F16)
                S1 = stg.tile([128, BC], F16)
                for b in range(B):
                    for half, S in ((0, S0), (1, S1)):
                        ps = tps.tile([128, 128], F16, tag="tps")
                        nc.tensor.transpose(
                            ps[:125, :CAT],
                            XT[:CAT, b * NOWN + half * 125:
                               b * NOWN + half * 125 + 125],
                            ident[:CAT, :CAT])
                        nc.vector.tensor_copy(
                            S[:125, :].rearrange("n (c b2) -> n c b2", b2=B)[:, :, b],
                            ps[:125, :CAT])
                nc.sync.dma_start(x1_own_d[:125, :], S0[:125, :])
                nc.sync.dma_start(x1_own_d[125:, :], S1[:125, :])

            # ---------- P3: AllGather x1 ----------
            if n_cores > 1:
                nc.gpsimd.collective_compute(
                    "AllGather", ALU.bypass,
                    replica_groups=[list(range(n_cores))],
                    ins=[x1_own_d[:]], outs=[x1_all_d[:]])
            else:
                nc.sync.dma_start(x1_all_d[:], x1_own_d[:])

            # ---------- P1: adaptive support, normalized to fp16 ----------
            with ExitStack() as p1:
                eT_pool = p1.enter_context(tc.tile_pool(name="eT", bufs=1))
                eT_all = eT_pool.tile([E, N], F32)
                nc.sync.dma_start(eT_all[:], eT_all_d[:])
                M_pool = p1.enter_context(tc.tile_pool(name="Msb", bufs=1))
                M_sb = M_pool.tile([128, MC * NOWN], BF16)
                rinv_bc = M_pool.tile([128, NOWN], F32)
                sm_pool = p1.enter_context(tc.tile_pool(name="sm", bufs=3))
                sm_psum = p1.enter_context(tc.tile_pool(name="sm_ps", bufs=2, space="PSUM"))
                for j, (m0, mp) in enumerate(mch):
                    ps = sm_psum.tile([128, NOWN], F32, tag="sm_ps")
                    nc.tensor.matmul(ps[:mp, :], eT_all[:, m0:m0 + mp],
                                     eT_own[:, :], start=True, stop=True)
                    ex = sm_pool.tile([128, NOWN], F32, tag="sm_ex")
                    nc.scalar.activation(ex[:mp, :], ps[:mp, :], AF.Exp)
                    nc.vector.tensor_scalar_max(M_sb[:mp, ts(j, NOWN)], ex[:mp, :],
                                                1.0)
                ones_pool = p1.enter_context(tc.tile_pool(name="ones", bufs=1))
                ones = ones_pool.tile([128, 1], BF16)
                nc.vector.memset(ones[:], 1.0)
                rs_ps = sm_psum.tile([1, NOWN], F32, tag="rs_ps")
                for j, (m0, mp) in enumerate(mch):
                    nc.tensor.matmul(rs_ps[:, :], ones[:mp, :],
                                     M_sb[:mp, ts(j, NOWN)],
                                     start=(j == 0), stop=(j == MC - 1))
                rinv = sm_pool.tile([1, NOWN], F32, tag="rinv")
                nc.vector.reciprocal(rinv[:, :], rs_ps[:, :])
                nc.gpsimd.partition_broadcast(rinv_bc[:], rinv[:, :])
                for j, (m0, mp) in enumerate(mch):
                    nc.vector.tensor_tensor(A_sb[:mp, ts(j, NOWN)],
                                            M_sb[:mp, ts(j, NOWN)],
                                            rinv_bc[:mp, :], ALU.mult)

            # ---------- P3b: W-gen (fp16) -> DRAM ----------
            nch = _chunks(NOWN, 128)
            with ExitStack() as p3:
                wgen_rhs = p3.enter_context(tc.tile_pool(name="wg_rhs", bufs=3))
                wgen_ps = p3.enter_context(tc.tile_pool(name="wg_ps", bufs=2, space="PSUM"))
                wgen_pool = p3.enter_context(tc.tile_pool(name="wg_ev", bufs=3))
                for (wdram_in, wdram, O) in ((wpg_d, wg_d, OG), (wpu_d, wu_d, OU)):
                    KO = KIALL * O
                    for (f0, fp) in _chunks(KO, 512):
                        rhs = wgen_rhs.tile([E, 512], F16, tag="wg_rhs")
                        nc.sync.dma_start(
                            rhs[:, :fp],
                            wdram_in[:].rearrange("e k o -> e (k o)")[:, f0:f0 + fp])
                        for (nn0, np_) in nch:
                            ps = wgen_ps.tile([128, 512], F32, tag="wg_ps")
                            nc.tensor.matmul(ps[:np_, :fp],
                                             eT16[:, nn0:nn0 + np_],
                                             rhs[:, :fp],
                                             start=True, stop=True)
                            ev = wgen_pool.tile([128, 512], F16, tag="wg_ev")
                            nc.vector.tensor_copy(ev[:np_, :fp], ps[:np_, :fp])
                            nc.sync.dma_start(
                                wdram[:].rearrange("n k o -> n (k o)")
                                [nn0:nn0 + np_, f0:f0 + fp],
                                ev[:np_, :fp])

            # ---------- conv helper: A @ X -> xg_d -> XG ----------
            def conv(parts, is_first):
                with ExitStack() as pc:
                    xg_pool = pc.enter_context(tc.tile_pool(name="xgrp", bufs=2))
                    cv_ps = pc.enter_context(tc.tile_pool(name="cv_ps", bufs=5, space="PSUM"))
                    cv_ev = pc.enter_context(tc.tile_pool(name="cv_ev", bufs=3))

                    def emit_q(q, lhs_tile, qi, gcols):
                        ps = cv_ps.tile([128, NOWN], F32, tag="cv_ps")
                        for j, (m0, mp) in enumerate(mch):
                            nc.tensor.matmul(
                                ps[:128, :],
                                lhs_tile[:mp, ts(j, gcols)][:, qi * 128:(qi + 1) * 128]
                                if gcols else lhs_tile[:mp, ts(j, 128)],
                                A_sb[:mp, ts(j, NOWN)],
                                start=(j == 0), stop=(j == MC - 1))
                        ev = cv_ev.tile([128, NOWN], F16, tag="cv_ev")
                        nc.vector.tensor_copy(ev[:, :], ps[:, :])
                        nc.sync.dma_start(xg_d[q * 128:(q + 1) * 128, :],
                                          ev[:, :])

                    for (dram, g0, gcols, qoffs) in parts:
                        if dram is None:
                            emit_q(qoffs[0], xcols, 0, 0)
                            continue
                        Xg = xg_pool.tile([128, MC * 5 * 128], F16, tag="xgrp")
                        for j, (m0, mp) in enumerate(mch):
                            nc.sync.dma_start(Xg[:mp, ts(j, gcols)],
                                              dram[m0:m0 + mp, g0:g0 + gcols])
                            if is_first and g0 + gcols == BC:
                                nc.vector.tensor_copy(
                                    xcols[:mp, ts(j, 128)],
                                    Xg[:mp, ts(j, gcols)][:, gcols - 128:])
                        for qi, q in enumerate(qoffs):
                            emit_q(q, Xg, qi, gcols)
                    nc.sync.dma_start(
                        XG[:].rearrange("c (b2 n) -> c b2 n", b2=B),
                        xg_d[:].rearrange("(c b2) n -> c b2 n", b2=B))

            def groups(q0, nq, cap=5):
                out = []
                q = q0
                while q < q0 + nq:
                    take = min(cap, q0 + nq - q)
                    out.append(list(range(q, q + take)))
                    q += take
                return out

            # ---------- P4: conv1 -> XG = xg1T ----------
            parts1 = [(x1_all_d, qs[0] * 128, len(qs) * 128, qs)
                      for qs in groups(0, 33)]
            conv(parts1, is_first=True)

            # ---------- P5: apply gate; spill z*state and r ----------
            with ExitStack() as p5:
                ap_w = p5.enter_context(tc.tile_pool(name="ap_w", bufs=2))
                ap_ps = p5.enter_context(tc.tile_pool(name="ap_ps", bufs=3, space="PSUM"))
                ap_t = p5.enter_context(tc.tile_pool(name="ap_t", bufs=3))
                for (nb0, nbp) in _chunks(NOWN, nblk):
                    w0 = ap_w.tile([KI0, nblk * OG], F16, tag="w0")
                    nc.sync.dma_start(
                        w0[:, :nbp * OG].rearrange("k (n o) -> k n o", o=OG),
                        wg_d[nb0:nb0 + nbp, :KI0, :].rearrange("n k o -> k n o"))
                    w1 = ap_w.tile([CAT, nblk * OG], F16, tag="w1")
                    nc.sync.dma_start(
                        w1[:, :nbp * OG].rearrange("k (n o) -> k n o", o=OG),
                        wg_d[nb0:nb0 + nbp, KI0:, :].rearrange("n k o -> k n o"))
                    for (g0, gp) in _chunks(nbp, 8):
                        ps_z = ap_ps.tile([H, 512], F32, tag="ap_psz")
                        ps_r = ap_ps.tile([H, 512], F32, tag="ap_psr")
                        for nl in range(g0, g0 + gp):
                            n = nb0 + nl
                            w = (nl - g0) * B
                            xt_n = XT[:, n::NOWN][:, :B]
                            xg_n = XG[:, n::NOWN][:, :B]
                            for ps, o0 in ((ps_z, 0), (ps_r, H)):
                                sl = slice(nl * OG + o0, nl * OG + o0 + H)
                                nc.tensor.matmul(ps[:, w:w + B], w0[:, sl],
                                                 xt_n, start=True, stop=False)
                                nc.tensor.matmul(ps[:, w:w + B], w1[:CAT, sl],
                                                 xg_n, start=False, stop=True)
                        cols = slice((nb0 + g0) * B, (nb0 + g0 + gp) * B)
                        zg = ap_t.tile([H, 512], F32, tag="zg")
                        nc.scalar.activation(zg[:, :gp * B], ps_z[:, :gp * B],
                                             AF.Sigmoid)
                        rg = ap_t.tile([H, 512], F16, tag="rg")
                        nc.scalar.activation(rg[:, :gp * B], ps_r[:, :gp * B],
                                             AF.Sigmoid)
                        nc.sync.dma_start(
                            r_d[nb0 + g0:nb0 + g0 + gp, :]
                            .rearrange("n (h b2) -> h n b2", b2=B),
                            rg[:, :gp * B].rearrange("h (n b2) -> h n b2", b2=B))
                        zs = ap_t.tile([H, 512], F16, tag="zs")
                        nc.vector.tensor_tensor(
                            zs[:, :gp * B].rearrange("h (n b2) -> h n b2", b2=B),
                            zg[:, :gp * B].rearrange("h (n b2) -> h n b2", b2=B),
                            XT[:H, :].rearrange("h (b2 n) -> h n b2", b2=B)
                            [:, nb0 + g0:nb0 + g0 + gp, :],
                            ALU.mult)
                        nc.sync.dma_start(
                            zs_own_d[nb0 + g0:nb0 + g0 + gp, :]
                            .rearrange("n (h b2) -> h n b2", b2=B),
                            zs[:, :gp * B].rearrange("h (n b2) -> h n b2", b2=B))

            # ---------- P6: AllGather z*state ----------
            if n_cores > 1:
                nc.gpsimd.collective_compute(
                    "AllGather", ALU.bypass,
                    replica_groups=[list(range(n_cores))],
                    ins=[zs_own_d[:]], outs=[zs_all_d[:]])
            else:
                nc.sync.dma_start(zs_all_d[:], zs_own_d[:])

            # ---------- P7: conv2 -> XG = xg2T ----------
            parts2 = [(zs_all_d, qs[0] * 128, len(qs) * 128, qs)
                      for qs in groups(0, 32)]
            parts2.append((None, 32 * 128, 128, [32]))
            conv(parts2, is_first=False)

            # ---------- P8: apply update; blend; output ----------
            with ExitStack() as p8:
                ap_w = p8.enter_context(tc.tile_pool(name="ap_w2", bufs=2))
                ap_ps = p8.enter_context(tc.tile_pool(name="ap_ps2", bufs=3, space="PSUM"))
                ap_t = p8.enter_context(tc.tile_pool(name="ap_t2", bufs=3))
                zst_pool = p8.enter_context(tc.tile_pool(name="zstb", bufs=2))
                for (nb0, nbp) in _chunks(NOWN, nblk):
                    w0a = ap_w.tile([H, nblk * OU], F16, tag="w0a")
                    nc.sync.dma_start(
                        w0a[:, :nbp * OU].rearrange("k (n o) -> k n o", o=OU),
                        wu_d[nb0:nb0 + nbp, :H, :].rearrange("n k o -> k n o"))
                    w0b = ap_w.tile([3, nblk * OU], F16, tag="w0b")
                    nc.sync.dma_start(
                        w0b[:, :nbp * OU].rearrange("k (n o) -> k n o", o=OU),
                        wu_d[nb0:nb0 + nbp, H:KI0, :].rearrange("n k o -> k n o"))
                    w1 = ap_w.tile([CAT, nblk * OU], F16, tag="w1u")
                    nc.sync.dma_start(
                        w1[:, :nbp * OU].rearrange("k (n o) -> k n o", o=OU),
                        wu_d[nb0:nb0 + nbp, KI0:, :].rearrange("n k o -> k n o"))
                    zst = zst_pool.tile([H, nblk * B], F16, tag="zstb")
                    nc.sync.dma_start(
                        zst[:, :nbp * B].rearrange("h (n b2) -> h n b2", b2=B),
                        zs_own_d[nb0:nb0 + nbp, :]
                        .rearrange("n (h b2) -> h n b2", b2=B))
                    rb = zst_pool.tile([H, nblk * B], F16, tag="rb")
                    nc.sync.dma_start(
                        rb[:, :nbp * B].rearrange("h (n b2) -> h n b2", b2=B),
                        r_d[nb0:nb0 + nbp, :]
                        .rearrange("n (h b2) -> h n b2", b2=B))
                    for (g0, gp) in _chunks(nbp, 8):
                        ps = ap_ps.tile([H, 512], F32, tag="ap_ps2")
                        for nl in range(g0, g0 + gp):
                            n = nb0 + nl
                            w = (nl - g0) * B
                            sl = slice(nl * OU, (nl + 1) * OU)
                            nc.tensor.matmul(ps[:, w:w + B], w0a[:, sl],
                                             zst[:, nl * B:(nl + 1) * B],
                                             start=True, stop=False)
                            nc.tensor.matmul(ps[:, w:w + B], w0b[:, sl],
                                             XB[:, n::NOWN][:, :B],
                                             start=False, stop=False)
                            nc.tensor.matmul(ps[:, w:w + B], w1[:CAT, sl],
                                             XG[:, n::NOWN][:, :B],
                                             start=False, stop=True)
                        cols = slice((nb0 + g0) * B, (nb0 + g0 + gp) * B)
                        hc = ap_t.tile([H, 512], F32, tag="hc")
                        nc.scalar.activation(hc[:, :gp * B], ps[:, :gp * B],
                                             AF.Tanh)
                        # out = hc + r*(state - hc)
                        t1 = ap_t.tile([H, 512], F32, tag="t1")
                        nc.vector.tensor_sub(
                            t1[:, :gp * B].rearrange("h (n b2) -> h n b2", b2=B),
                            XT[:H, :].rearrange("h (b2 n) -> h n b2", b2=B)
                            [:, nb0 + g0:nb0 + g0 + gp, :],
                            hc[:, :gp * B].rearrange("h (n b2) -> h n b2", b2=B))
                        t2 = ap_t.tile([H, 512], F32, tag="t2")
                        nc.vector.tensor_tensor(
                            t2[:, :gp * B], t1[:, :gp * B],
                            rb[:, cols.start - nb0 * B:cols.stop - nb0 * B],
                            ALU.mult)
                        ot = ap_t.tile([H, 512], F16, tag="ot")
                        nc.vector.tensor_add(ot[:, :gp * B], t2[:, :gp * B],
                                             hc[:, :gp * B])
                        nc.sync.dma_start(out_d[:, cols], ot[:, :gp * B])

    nc.compile()
    return nc


_NC_CACHE = {}


def _get_nc(n_cores=8):
    if n_cores not in _NC_CACHE:
        _NC_CACHE[n_cores] = build_nc(n_cores=n_cores)
    return _NC_CACHE[n_cores]


def _pack_pool(wp, bias, O):
    """[E,K,CAT,O] pool + [E,O] bias -> [E, 133, O] fp16.

    Row order per k-slab: (state rows, x rows); bias at row 66."""
    out = np.empty((E, KIALL, O), np.float32)
    out[:, :H, :] = wp[:, 0, DIN:, :]
    out[:, H:CAT, :] = wp[:, 0, :DIN, :]
    out[:, CAT, :] = bias
    out[:, KI0:KI0 + H, :] = wp[:, 1, DIN:, :]
    out[:, KI0 + H:, :] = wp[:, 1, :DIN, :]
    return out.astype(np.float16)


def _build_in_maps(x, state, node_embed, gate_weights_pool, gate_bias_pool,
                   update_weights_pool, update_bias_pool, n_cores=8):
    x = np.asarray(x, np.float32)
    state = np.asarray(state, np.float32)
    node_embed = np.asarray(node_embed, np.float32)
    eT = np.ascontiguousarray(node_embed.T)                 # [E, N]
    x1_nbc = np.concatenate([state.transpose(1, 0, 2), x.transpose(1, 0, 2)],
                            axis=2)                          # [N, B, CAT]
    wpg = _pack_pool(np.asarray(gate_weights_pool, np.float32),
                     np.asarray(gate_bias_pool, np.float32), OG)
    wpu = _pack_pool(np.asarray(update_weights_pool, np.float32),
                     np.asarray(update_bias_pool, np.float32), OU)
    in_maps = []
    for c in range(n_cores):
        sl = slice(c * NOWN, (c + 1) * NOWN)
        eT_own = np.ascontiguousarray(eT[:, sl])
        xt = np.ones((KI0, B, NOWN), np.float32)
        xt[:CAT] = x1_nbc[sl].transpose(2, 1, 0)
        in_maps.append({
            "xt_in": xt.astype(np.float16).reshape(KI0, NB),
            "eT_all": eT,
            "eT_own": eT_own,
            "wpg": wpg,
            "wpu": wpu,
        })
    return in_maps


def kernel(x, state, node_embed, gate_weights_pool, gate_bias_pool,
           update_weights_pool, update_bias_pool, n_cores=8):
    nc = _get_nc(n_cores)
    in_maps = _build_in_maps(x, state, node_embed, gate_weights_pool,
                             gate_bias_pool, update_weights_pool,
                             update_bias_pool, n_cores)
    res = run_bass_kernel_spmd(nc, in_maps, list(range(n_cores)))
    outs = []
    for c in range(n_cores):
        o = np.asarray(res.results[c]["out"], dtype=np.float32)  # [H, NOWN*B]
        outs.append(o.reshape(H, NOWN, B).transpose(2, 1, 0))    # [B, NOWN, H]
    return np.concatenate(outs, axis=1)


# revision 5
# speedup vs baseline: 1.0542x; 1.0507x over previous
"""Trainium2 Bass kernel for FMGCNCell (adaptive-graph GRU cell), v3 (fp16).

Per-call host->device staging dominates (~0.53 ms/MB/core), so each core
ships only its own node slab (~3 MB) and the full X matrix is assembled
on-device via AllGather. The whole datapath runs in fp16 (10-bit mantissa)
with f32 PSUM accumulation; the adaptive support is pre-normalized so its
fp16 copy stays in [0, 1]. Output returns fp16, host converts.

Sharding: node-parallel (each core owns N/8 = 250 output nodes, full batch).
"""

from contextlib import ExitStack

import numpy as np
import ml_dtypes

import concourse.bass as bass
import concourse.bacc as bacc
import concourse.tile as tile
from concourse import mybir
from concourse.bass import ds, ts
from concourse.bass_utils import run_bass_kernel_spmd
from concourse.masks import make_identity

F32 = mybir.dt.float32
BF16 = mybir.dt.bfloat16
F16 = mybir.dt.float16
AF = mybir.ActivationFunctionType
ALU = mybir.AluOpType

B = 64
DIN = 2
H = 64
E = 16
CAT = DIN + H            # 66
KI0 = CAT + 1            # 67
KIALL = 2 * CAT + 1      # 133
OG = 2 * H               # 128
OU = H                   # 64
BC = B * CAT             # 4224; X layout (c, b) c-major
N = 2000
NC_ = 8
NOWN = N // NC_          # 250
NB = B * NOWN            # 16000


def _chunks(total, size):
    out = []
    off = 0
    while off < total:
        out.append((off, min(size, total - off)))
        off += size
    return out


def build_nc(n_cores=8, nblk=25):
    mch = _chunks(N, 128)
    MC = len(mch)

    nc = bacc.Bacc("TRN2", target_bir_lowering=False, debug=False,
                   num_devices=n_cores)

    # ---- external inputs ----
    xt_d = nc.dram_tensor("xt_in", [KI0, NB], F16, kind="ExternalInput")
    eT_all_d = nc.dram_tensor("eT_all", [E, N], F32, kind="ExternalInput")
    eT_own_d = nc.dram_tensor("eT_own", [E, NOWN], F32, kind="ExternalInput")
    wpg_d = nc.dram_tensor("wpg", [E, KIALL, OG], F16, kind="ExternalInput")
    wpu_d = nc.dram_tensor("wpu", [E, KIALL, OU], F16, kind="ExternalInput")

    # output: [h, (n, b)] n-major, fp16 (host reassembles)
    out_d = nc.dram_tensor("out", [H, NB], F16, kind="ExternalOutput")

    # ---- internal DRAM ----
    x1_own_d = nc.dram_tensor("x1_own", [NOWN, BC], F16)
    zs_own_d = nc.dram_tensor("zs_own", [NOWN, H * B], F16)
    wg_d = nc.dram_tensor("wg_d", [KIALL, NOWN, OG], F16)
    wu_d = nc.dram_tensor("wu_d", [KIALL, NOWN, OU], F16)
    xg_d = nc.dram_tensor("xg_d", [BC, NOWN], F16)
    r_d = nc.dram_tensor("r_d", [NOWN, H * B], F16)
    shared = dict(addr_space="Shared") if n_cores > 1 else {}
    x1_all_d = nc.dram_tensor("x1_all", [N, BC], F16, **shared)
    zs_all_d = nc.dram_tensor("zs_all", [N, H * B], F16, **shared)

    with tile.TileContext(nc) as tc:
        with ExitStack() as root:
            persist = root.enter_context(tc.tile_pool(name="persist", bufs=1))
            XT = persist.tile([KI0, NB], F16)            # [c, (b, n)] b-major
            A_sb = persist.tile([128, MC * NOWN], F16)   # normalized support
            xcols = persist.tile([128, MC * 128], F16)   # X cols 4096:4224
            XG = persist.tile([CAT, NB], F16)            # xg1T then xg2T
            ident = persist.tile([128, 128], F16)
            make_identity(nc, ident[:])
            XB = persist.tile([3, NB], F16)              # x|ones rows at part 0
            eT_own = persist.tile([E, NOWN], F32)
            eT16 = persist.tile([E, NOWN], F16)

            # ---------- P0: load inputs ----------
            nc.sync.dma_start(XT[:], xt_d[:])
            nc.sync.dma_start(XB[:], xt_d[H:KI0, :])
            nc.sync.dma_start(eT_own[:], eT_own_d[:])
            nc.vector.tensor_copy(eT16[:], eT_own[:])

            # ---------- P2: build x1_own from XT by transposes ----------
            with ExitStack() as p2:
                tps = p2.enter_context(tc.tile_pool(name="tps", bufs=3, space="PSUM"))
                stg = p2.enter_context(tc.tile_pool(name="stg", bufs=1))
                S0 = stg.tile([128, BC], F16)
                S1 = stg.tile([128, BC], F16)
                for b in range(B):
                    for half, S in ((0, S0), (1, S1)):
                        ps = tps.tile([128, 128], F16, tag="tps")
                        nc.tensor.transpose(
                            ps[:125, :CAT],
                            XT[:CAT, b * NOWN + half * 125:
                               b * NOWN + half * 125 + 125],
                            ident[:CAT, :CAT])
                        nc.vector.tensor_copy(
                            S[:125, :].rearrange("n (c b2) -> n c b2", b2=B)[:, :, b],
                            ps[:125, :CAT])
                nc.sync.dma_start(x1_own_d[:125, :], S0[:125, :])
                nc.sync.dma_start(x1_own_d[125:, :], S1[:125, :])

            # ---------- P3: AllGather x1 ----------
            if n_cores > 1:
                nc.gpsimd.collective_compute(
                    "AllGather", ALU.bypass,
                    replica_groups=[list(range(n_cores))],
                    ins=[x1_own_d[:]], outs=[x1_all_d[:]])
            else:
                nc.sync.dma_start(x1_all_d[:], x1_own_d[:])

            # ---------- P1: adaptive support, normalized to fp16 ----------
            with ExitStack() as p1:
                eT_pool = p1.enter_context(tc.tile_pool(name="eT", bufs=1))
                eT_all = eT_pool.tile([E, N], F32)
                nc.sync.dma_start(eT_all[:], eT_all_d[:])
                M_pool = p1.enter_context(tc.tile_pool(name="Msb", bufs=1))
                M_sb = M_pool.tile([128, MC * NOWN], BF16)
                rinv_bc = M_pool.tile([128, NOWN], F32)
                sm_pool = p1.enter_context(tc.tile_pool(name="sm", bufs=3))
                sm_psum = p1.enter_context(tc.tile_pool(name="sm_ps", bufs=2, space="PSUM"))
                for j, (m0, mp) in enumerate(mch):
                    ps = sm_psum.tile([128, NOWN], F32, tag="sm_ps")
                    nc.tensor.matmul(ps[:mp, :], eT_all[:, m0:m0 + mp],
                                     eT_own[:, :], start=True, stop=True)
                    ex = sm_pool.tile([128, NOWN], F32, tag="sm_ex")
                    nc.scalar.activation(ex[:mp, :], ps[:mp, :], AF.Exp)
                    nc.vector.tensor_scalar_max(M_sb[:mp, ts(j, NOWN)], ex[:mp, :],
                                                1.0)
                ones_pool = p1.enter_context(tc.tile_pool(name="ones", bufs=1))
                ones = ones_pool.tile([128, 1], BF16)
                nc.vector.memset(ones[:], 1.0)
                rs_ps = sm_psum.tile([1, NOWN], F32, tag="rs_ps")
                for j, (m0, mp) in enumerate(mch):
                    nc.tensor.matmul(rs_ps[:, :], ones[:mp, :],
                                     M_sb[:mp, ts(j, NOWN)],
                                     start=(j == 0), stop=(j == MC - 1))
                rinv = sm_pool.tile([1, NOWN], F32, tag="rinv")
                nc.vector.reciprocal(rinv[:, :], rs_ps[:, :])
                nc.gpsimd.partition_broadcast(rinv_bc[:], rinv[:, :])
                for j, (m0, mp) in enumerate(mch):
                    nc.vector.tensor_tensor(A_sb[:mp, ts(j, NOWN)],
                                            M_sb[:mp, ts(j, NOWN)],
                                            rinv_bc[:mp, :], ALU.mult)

            # ---------- P3b: W-gen (fp16) -> DRAM ----------
            nch = _chunks(NOWN, 128)
            with ExitStack() as p3:
                wgen_rhs = p3.enter_context(tc.tile_pool(name="wg_rhs", bufs=3))
                wgen_ps = p3.enter_context(tc.tile_pool(name="wg_ps", bufs=2, space="PSUM"))
                wgen_pool = p3.enter_context(tc.tile_pool(name="wg_ev", bufs=3))
                for (wdram_in, wdram, O) in ((wpg_d, wg_d, OG), (wpu_d, wu_d, OU)):
                    KO = KIALL * O
                    for (f0, fp) in _chunks(KO, 512):
                        rhs = wgen_rhs.tile([E, 512], F16, tag="wg_rhs")
                        nc.sync.dma_start(
                            rhs[:, :fp],
                            wdram_in[:].rearrange("e k o -> e (k o)")[:, f0:f0 + fp])
                        for (nn0, np_) in nch:
                            ps = wgen_ps.tile([128, 512], F32, tag="wg_ps")
                            nc.tensor.matmul(ps[:np_, :fp],
                                             eT16[:, nn0:nn0 + np_],
                                             rhs[:, :fp],
                                             start=True, stop=True)
                            ev = wgen_pool.tile([128, 512], F16, tag="wg_ev")
                            nc.vector.tensor_copy(ev[:np_, :fp], ps[:np_, :fp])
                            nki = fp // O
                            nc.sync.dma_start(
                                wdram[f0 // O:f0 // O + nki,
                                      nn0:nn0 + np_, :]
                                .rearrange("k n o -> n k o"),
                                ev[:np_, :fp]
                                .rearrange("n (k o) -> n k o", o=O))

            # ---------- conv helper: A @ X -> xg_d -> XG ----------
            def conv(parts, is_first):
                with ExitStack() as pc:
                    xg_pool = pc.enter_context(tc.tile_pool(name="xgrp", bufs=2))
                    cv_ps = pc.enter_context(tc.tile_pool(name="cv_ps", bufs=5, space="PSUM"))
                    cv_ev = pc.enter_context(tc.tile_pool(name="cv_ev", bufs=3))

                    def emit_q(q, lhs_tile, qi, gcols):
                        ps = cv_ps.tile([128, NOWN], F32, tag="cv_ps")
                        for j, (m0, mp) in enumerate(mch):
                            nc.tensor.matmul(
                                ps[:128, :],
                                lhs_tile[:mp, ts(j, gcols)][:, qi * 128:(qi + 1) * 128]
                                if gcols else lhs_tile[:mp, ts(j, 128)],
                                A_sb[:mp, ts(j, NOWN)],
                                start=(j == 0), stop=(j == MC - 1))
                        ev = cv_ev.tile([128, NOWN], F16, tag="cv_ev")
                        nc.vector.tensor_copy(ev[:, :], ps[:, :])
                        nc.sync.dma_start(xg_d[q * 128:(q + 1) * 128, :],
                                          ev[:, :])

                    for (dram, g0, gcols, qoffs) in parts:
                        if dram is None:
                            emit_q(qoffs[0], xcols, 0, 0)
                            continue
                        Xg = xg_pool.tile([128, MC * 5 * 128], F16, tag="xgrp")
                        for j, (m0, mp) in enumerate(mch):
                            nc.sync.dma_start(Xg[:mp, ts(j, gcols)],
                                              dram[m0:m0 + mp, g0:g0 + gcols])
                            if is_first and g0 + gcols == BC:
                                nc.vector.tensor_copy(
                                    xcols[:mp, ts(j, 128)],
                                    Xg[:mp, ts(j, gcols)][:, gcols - 128:])
                        for qi, q in enumerate(qoffs):
                            emit_q(q, Xg, qi, gcols)
                    nc.sync.dma_start(
                        XG[:].rearrange("c (b2 n) -> c b2 n", b2=B),
                        xg_d[:].rearrange("(c b2) n -> c b2 n", b2=B))

            def groups(q0, nq, cap=5):
                out = []
                q = q0
                while q < q0 + nq:
                    take = min(cap, q0 + nq - q)
                    out.append(list(range(q, q + take)))
                    q += take
                return out

            # ---------- P4: conv1 -> XG = xg1T ----------
            parts1 = [(x1_all_d, qs[0] * 128, len(qs) * 128, qs)
                      for qs in groups(0, 33)]
            conv(parts1, is_first=True)

            # ---------- P5: apply gate; spill z*state and r ----------
            with ExitStack() as p5:
                ap_w = p5.enter_context(tc.tile_pool(name="ap_w", bufs=2))
                ap_ps = p5.enter_context(tc.tile_pool(name="ap_ps", bufs=3, space="PSUM"))
                ap_t = p5.enter_context(tc.tile_pool(name="ap_t", bufs=3))
                for (nb0, nbp) in _chunks(NOWN, nblk):
                    w0 = ap_w.tile([KI0, nblk * OG], F16, tag="w0")
                    nc.sync.dma_start(
                        w0[:, :nbp * OG].rearrange("k (n o) -> k n o", o=OG),
                        wg_d[:KI0, nb0:nb0 + nbp, :])
                    w1 = ap_w.tile([CAT, nblk * OG], F16, tag="w1")
                    nc.sync.dma_start(
                        w1[:, :nbp * OG].rearrange("k (n o) -> k n o", o=OG),
                        wg_d[KI0:, nb0:nb0 + nbp, :])
                    for (g0, gp) in _chunks(nbp, 8):
                        ps_z = ap_ps.tile([H, 512], F32, tag="ap_psz")
                        ps_r = ap_ps.tile([H, 512], F32, tag="ap_psr")
                        for nl in range(g0, g0 + gp):
                            n = nb0 + nl
                            w = (nl - g0) * B
                            xt_n = XT[:, n::NOWN][:, :B]
                            xg_n = XG[:, n::NOWN][:, :B]
                            for ps, o0 in ((ps_z, 0), (ps_r, H)):
                                sl = slice(nl * OG + o0, nl * OG + o0 + H)
                                nc.tensor.matmul(ps[:, w:w + B], w0[:, sl],
                                                 xt_n, start=True, stop=False)
                                nc.tensor.matmul(ps[:, w:w + B], w1[:CAT, sl],
                                                 xg_n, start=False, stop=True)
                        cols = slice((nb0 + g0) * B, (nb0 + g0 + gp) * B)
                        zg = ap_t.tile([H, 512], F32, tag="zg")
                        nc.scalar.activation(zg[:, :gp * B], ps_z[:, :gp * B],
                                             AF.Sigmoid)
                        rg = ap_t.tile([H, 512], F16, tag="rg")
                        nc.scalar.activation(rg[:, :gp * B], ps_r[:, :gp * B],
                                             AF.Sigmoid)
                        nc.sync.dma_start(
                            r_d[nb0 + g0:nb0 + g0 + gp, :]
                            .rearrange("n (h b2) -> h n b2", b2=B),
                            rg[:, :gp * B].rearrange("h (n b2) -> h n b2", b2=B))
                        zs = ap_t.tile([H, 512], F16, tag="zs")
                        nc.vector.tensor_tensor(
                            zs[:, :gp * B].rearrange("h (n b2) -> h n b2", b2=B),
                            zg[:, :gp * B].rearrange("h (n b2) -> h n b2", b2=B),
                            XT[:H, :].rearrange("h (b2 n) -> h n b2", b2=B)
                            [:, nb0 + g0:nb0 + g0 + gp, :],
                            ALU.mult)
                        nc.sync.dma_start(
                            zs_own_d[nb0 + g0:nb0 + g0 + gp, :]
                            .rearrange("n (h b2) -> h n b2", b2=B),
                            zs[:, :gp * B].rearrange("h (n b2) -> h n b2", b2=B))

            # ---------- P6: AllGather z*state ----------
            if n_cores > 1:
                nc.gpsimd.collective_compute(
                    "AllGather", ALU.bypass,
                    replica_groups=[list(range(n_cores))],
                    ins=[zs_own_d[:]], outs=[zs_all_d[:]])
            else:
                nc.sync.dma_start(zs_all_d[:], zs_own_d[:])

            # ---------- P7: conv2 -> XG = xg2T ----------
            parts2 = [(zs_all_d, qs[0] * 128, len(qs) * 128, qs)
                      for qs in groups(0, 32)]
            parts2.append((None, 32 * 128, 128, [32]))
            conv(parts2, is_first=False)

            # ---------- P8: apply update; blend; output ----------
            with ExitStack() as p8:
                ap_w = p8.enter_context(tc.tile_pool(name="ap_w2", bufs=2))
                ap_ps = p8.enter_context(tc.tile_pool(name="ap_ps2", bufs=3, space="PSUM"))
                ap_t = p8.enter_context(tc.tile_pool(name="ap_t2", bufs=3))
                zst_pool = p8.enter_context(tc.tile_pool(name="zstb", bufs=2))
                for (nb0, nbp) in _chunks(NOWN, nblk):
                    w0a = ap_w.tile([H, nblk * OU], F16, tag="w0a")
                    nc.sync.dma_start(
                        w0a[:, :nbp * OU].rearrange("k (n o) -> k n o", o=OU),
                        wu_d[:H, nb0:nb0 + nbp, :])
                    w0b = ap_w.tile([3, nblk * OU], F16, tag="w0b")
                    nc.sync.dma_start(
                        w0b[:, :nbp * OU].rearrange("k (n o) -> k n o", o=OU),
                        wu_d[H:KI0, nb0:nb0 + nbp, :])
                    w1 = ap_w.tile([CAT, nblk * OU], F16, tag="w1u")
                    nc.sync.dma_start(
                        w1[:, :nbp * OU].rearrange("k (n o) -> k n o", o=OU),
                        wu_d[KI0:, nb0:nb0 + nbp, :])
                    zst = zst_pool.tile([H, nblk * B], F16, tag="zstb")
                    nc.sync.dma_start(
                        zst[:, :nbp * B].rearrange("h (n b2) -> h n b2", b2=B),
                        zs_own_d[nb0:nb0 + nbp, :]
                        .rearrange("n (h b2) -> h n b2", b2=B))
                    rb = zst_pool.tile([H, nblk * B], F16, tag="rb")
                    nc.sync.dma_start(
                        rb[:, :nbp * B].rearrange("h (n b2) -> h n b2", b2=B),
                        r_d[nb0:nb0 + nbp, :]
                        .rearrange("n (h b2) -> h n b2", b2=B))
                    for (g0, gp) in _chunks(nbp, 8):
                        ps = ap_ps.tile([H, 512], F32, tag="ap_ps2")
                        for nl in range(g0, g0 + gp):
                            n = nb0 + nl
                            w = (nl - g0) * B
                            sl = slice(nl * OU, (nl + 1) * OU)
                            nc.tensor.matmul(ps[:, w:w + B], w0a[:, sl],
                                             zst[:, nl * B:(nl + 1) * B],
                                             start=True, stop=False)
                            nc.tensor.matmul(ps[:, w:w + B], w0b[:, sl],
                                             XB[:, n::NOWN][:, :B],
                                             start=False, stop=False)
                            nc.tensor.matmul(ps[:, w:w + B], w1[:CAT, sl],
                                             XG[:, n::NOWN][:, :B],
                                             start=False, stop=True)
                        cols = slice((nb0 + g0) * B, (nb0 + g0 + gp) * B)
                        hc = ap_t.tile([H, 512], F32, tag="hc")
                        nc.scalar.activation(hc[:, :gp * B], ps[:, :gp * B],
                                             AF.Tanh)
                        # out = hc + r*(state - hc)
                        t1 = ap_t.tile([H, 512], F32, tag="t1")
                        nc.vector.tensor_sub(
                            t1[:, :gp * B].rearrange("h (n b2) -> h n b2", b2=B),
                            XT[:H, :].rearrange("h (b2 n) -> h n b2", b2=B)
                            [:, nb0 + g0:nb0 + g0 + gp, :],
                            hc[:, :gp * B].rearrange("h (n b2) -> h n b2", b2=B))
                        t2 = ap_t.tile([H, 512], F32, tag="t2")
                        nc.vector.tensor_tensor(
                            t2[:, :gp * B], t1[:, :gp * B],
                            rb[:, cols.start - nb0 * B:cols.stop - nb0 * B],
                            ALU.mult)
                        ot = ap_t.tile([H, 512], F16, tag="ot")
                        nc.vector.tensor_add(ot[:, :gp * B], t2[:, :gp * B],
                                             hc[:, :gp * B])
                        nc.sync.dma_start(out_d[:, cols], ot[:, :gp * B])

    nc.compile()
    return nc


_NC_CACHE = {}


def _get_nc(n_cores=8):
    if n_cores not in _NC_CACHE:
        _NC_CACHE[n_cores] = build_nc(n_cores=n_cores)
    return _NC_CACHE[n_cores]


def _pack_pool(wp, bias, O):
    """[E,K,CAT,O] pool + [E,O] bias -> [E, 133, O] fp16.

    Row order per k-slab: (state rows, x rows); bias at row 66."""
    out = np.empty((E, KIALL, O), np.float32)
    out[:, :H, :] = wp[:, 0, DIN:, :]
    out[:, H:CAT, :] = wp[:, 0, :DIN, :]
    out[:, CAT, :] = bias
    out[:, KI0:KI0 + H, :] = wp[:, 1, DIN:, :]
    out[:, KI0 + H:, :] = wp[:, 1, :DIN, :]
    return out.astype(np.float16)


def _build_in_maps(x, state, node_embed, gate_weights_pool, gate_bias_pool,
                   update_weights_pool, update_bias_pool, n_cores=8):
    x = np.asarray(x, np.float32)
    state = np.asarray(state, np.float32)
    node_embed = np.asarray(node_embed, np.float32)
    eT = np.ascontiguousarray(node_embed.T)                 # [E, N]
    x1_nbc = np.concatenate([state.transpose(1, 0, 2), x.transpose(1, 0, 2)],
                            axis=2)                          # [N, B, CAT]
    wpg = _pack_pool(np.asarray(gate_weights_pool, np.float32),
                     np.asarray(gate_bias_pool, np.float32), OG)
    wpu = _pack_pool(np.asarray(update_weights_pool, np.float32),
                     np.asarray(update_bias_pool, np.float32), OU)
    in_maps = []
    for c in range(n_cores):
        sl = slice(c * NOWN, (c + 1) * NOWN)
        eT_own = np.ascontiguousarray(eT[:, sl])
        xt = np.ones((KI0, B, NOWN), np.float32)
        xt[:CAT] = x1_nbc[sl].transpose(2, 1, 0)
        in_maps.append({
            "xt_in": xt.astype(np.float16).reshape(KI0, NB),
            "eT_all": eT,
            "eT_own": eT_own,
            "wpg": wpg,
            "wpu": wpu,
        })
    return in_maps


def kernel(x, state, node_embed, gate_weights_pool, gate_bias_pool,
           update_weights_pool, update_bias_pool, n_cores=8):
    nc = _get_nc(n_cores)
    in_maps = _build_in_maps(x, state, node_embed, gate_weights_pool,
                             gate_bias_pool, update_weights_pool,
                             update_bias_pool, n_cores)
    res = run_bass_kernel_spmd(nc, in_maps, list(range(n_cores)))
    outs = []
    for c in range(n_cores):
        o = np.asarray(res.results[c]["out"], dtype=np.float32)  # [H, NOWN*B]
        outs.append(o.reshape(H, NOWN, B).transpose(2, 1, 0))    # [B, NOWN, H]
    return np.concatenate(outs, axis=1)
